# revision 31
# baseline (speedup 1.0000x reference)
"""Trainium2 Bass kernel for MaxViT-style grid-attention block.

Full module: x -> LN1 -> grid-partition attention (8 heads, 80-token
windows) -> layerscale residual -> LN2 -> MLP(256->1024 GELU ->256) ->
layerscale residual.

Sharding: data-parallel over batch B=16 across 8 cores (2 batch elems
per core); weights replicated.

Per-core dataflow (per batch element, 64 windows x 80 tokens):
  - x loaded window-gathered into "window-column" layout [80, 64, 256]
    (partition = token-in-window, free = (window, channel)).
  - LN1 stats via bn_stats; apply via two broadcast tensor_tensor ops
    (gamma/beta folded into weights/biases on host).
  - h transposed per-window to feature-major via PE transposes.
  - QKV: q,k via standard fm matmuls; v via flipped matmuls directly
    into token-major [80, 256] per window (+ ones column for the
    softmax denominator trick).
  - S' = k^T q per (window, head) -> [keys, q] in PSUM; exp on ACT
    (scale folded); PV with E as stationary and [v | 1] as moving gives
    O_tm [80q, 33] with the denominator in column 32. Normalize with
    per-partition reciprocal on eviction.
  - proj flipped (stationary = O_fm) to produce token-major proj out,
    residual-added in place into x (layerscale folded into proj_w).
  - LN2 same as LN1; MLP feature-major; fc2 output transposed back and
    residual-added in place (layerscale folded into fc2_w).
"""

import os
import sys

sys.path.insert(0, "/opt/trn_rl_repo")

KSTAGE = int(os.environ.get("KSTAGE", "4"))
KATTN = int(os.environ.get("KATTN", "3"))

import numpy as np
import ml_dtypes

import concourse.bass as bass
import concourse.bacc as bacc
import concourse.tile as tile
from concourse import mybir
from concourse import bass_utils
from concourse.masks import make_identity

F32 = mybir.dt.float32
BF16 = mybir.dt.bfloat16
AF = mybir.ActivationFunctionType
ALU = mybir.AluOpType

# Problem constants (hardcoded per contract)
B, H, W, C = 16, 64, 80, 256
GH, GW = 8, 10
HEADS, DH = 8, 32
INNER = 1024
SCALE = DH**-0.5
EPS = 1e-5

NCORES = 8
B_LOC = B // NCORES          # 2 batch elems per core
NWIN = (H // GH) * (W // GW)  # 64 windows per batch elem
NT = GH * GW                  # 80 tokens per window
NTOK = NWIN * NT              # 5120 tokens per batch elem
WBLK = 4                      # windows per token-block (320 tokens)
NBLK = NWIN // WBLK           # 16 token-blocks
BLKTOK = WBLK * NT            # 320


def _bf16(a):
    return np.asarray(a, np.float32).astype(ml_dtypes.bfloat16)


def build_nc():
    nc = bacc.Bacc("TRN2", target_bir_lowering=False, debug=False,
                   enable_asserts=False)

    # ---- DRAM I/O (per-core shapes) ----
    x_d = nc.dram_tensor("x", [B_LOC, H, W, C], F32, kind="ExternalInput")
    out_d = nc.dram_tensor("out", [B_LOC, H, W, C], F32, kind="ExternalOutput")
    wqk_d = nc.dram_tensor("wqk", [2, 128, 512], BF16, kind="ExternalInput")
    wv_d = nc.dram_tensor("wv", [2, 128, 256], BF16, kind="ExternalInput")
    wp_d = nc.dram_tensor("wp", [2, 128, 256], BF16, kind="ExternalInput")
    wf1_d = nc.dram_tensor("wf1", [2, 128, INNER], BF16, kind="ExternalInput")
    wf2_d = nc.dram_tensor("wf2", [8, 128, 256], BF16, kind="ExternalInput")

    # window-gathered views of x / out:
    # [b, (gh hh), (gw ww), c] -> [b, gh, gw, (hh ww), c]
    x_g = x_d.ap().rearrange("b (gh hh) (gw ww) c -> b gh gw hh ww c",
                             gh=GH, gw=GW)
    out_g = out_d.ap().rearrange("b (gh hh) (gw ww) c -> b gh gw hh ww c",
                                 gh=GH, gw=GW)

    with tile.TileContext(nc) as tc:
        consts = tc.alloc_tile_pool(name="consts", bufs=1)
        pool_x = tc.alloc_tile_pool(name="x", bufs=2)
        pool_ln = tc.alloc_tile_pool(name="ln", bufs=4)
        pool_fm = tc.alloc_tile_pool(name="fm", bufs=4)
        pool_qk = tc.alloc_tile_pool(name="qk", bufs=2)
        pool_v = tc.alloc_tile_pool(name="v", bufs=6)
        pool_e = tc.alloc_tile_pool(name="e", bufs=10)
        pool_ot = tc.alloc_tile_pool(name="ot", bufs=12)
        pool_of = tc.alloc_tile_pool(name="of", bufs=4)
        pool_g = tc.alloc_tile_pool(name="g", bufs=3)
        pool_f2 = tc.alloc_tile_pool(name="f2", bufs=4)
        pool_st = tc.alloc_tile_pool(name="st", bufs=3)
        psum_big = tc.alloc_tile_pool(name="pbig", bufs=2, space="PSUM")
        psum_acc = tc.alloc_tile_pool(name="pacc", bufs=1, space="PSUM")
        psum_sm = tc.alloc_tile_pool(name="psm", bufs=4, space="PSUM")
        psum_tr = tc.alloc_tile_pool(name="ptr", bufs=1, space="PSUM")

        # ---- constants ----
        id128 = consts.tile([128, 128], BF16)
        make_identity(nc, id128)
        eps_sb = consts.tile([128, 1], F32)
        nc.gpsimd.memset(eps_sb, EPS)

        def load_w(dram_ap, n, shape, nm):
            ts = []
            for i in range(n):
                t = consts.tile(shape, BF16, name=f"{nm}{i}")
                nc.sync.dma_start(out=t, in_=dram_ap[i])
                ts.append(t)
            return ts

        wqk_sb = load_w(wqk_d.ap(), 2, [128, 512], "wqk")
        wv_sb = load_w(wv_d.ap(), 2, [128, 256], "wv")
        wp_sb = load_w(wp_d.ap(), 2, [128, 256], "wp")
        wf1_sb = load_w(wf1_d.ap(), 2, [128, INNER], "wf1")
        wf2_sb = load_w(wf2_d.ap(), 8, [128, 256], "wf2")

        NWC = 32            # windows per chunk (half a batch elem)
        NTOKC = NWC * NT    # 2560
        NBLKC = NWC // WBLK  # 8
        GW_W = GH           # hh count per half = NWC // GW_W = 4

        def emit_store(b, hh0, x_wc4):
            for gh in range(GH):
                nc.sync.dma_start(
                    out=out_g[b, gh][:, hh0:hh0 + NWC // GW_W],
                    in_=x_wc4[gh * GW:(gh + 1) * GW])

        def emit_ln(x_wc):
            """x_wc [80, 64, 256] f32 -> per-token (mean, 1/std as bf16);
            gamma/beta folded into downstream weights."""
            st6 = pool_st.tile([80, NWC, 6], F32, tag="st6")
            for w0 in range(NWC):
                nc.vector.bn_stats(st6[:, w0], x_wc[:, w0])
            m = pool_st.tile([80, NWC], F32, tag="m")
            var = pool_st.tile([80, NWC], F32, tag="var")
            t0 = pool_st.tile([80, NWC], F32, tag="t0")
            t1 = pool_st.tile([80, NWC], F32, tag="t1")
            # mean = (m_even + m_odd) / 2
            nc.vector.tensor_tensor(t0, st6[:, :, 1], st6[:, :, 4], ALU.add)
            nc.vector.tensor_scalar(m, t0, 0.5, None, ALU.mult)
            # var = (cv_e + cv_o)/256 + ((m_e - m_o)/2)^2
            nc.vector.tensor_tensor(t0, st6[:, :, 2], st6[:, :, 5], ALU.add)
            nc.vector.tensor_tensor(t1, st6[:, :, 1], st6[:, :, 4],
                                    ALU.subtract)
            nc.vector.tensor_tensor(t1, t1, t1, ALU.mult)
            nc.vector.tensor_scalar(t0, t0, 1.0 / C, None, ALU.mult)
            nc.vector.tensor_scalar(t1, t1, 0.25, None, ALU.mult)
            nc.vector.tensor_tensor(var, t0, t1, ALU.add)
            # r = rsqrt(var + eps) = exp(-0.5 * ln(var + eps))
            lnv = pool_st.tile([80, NWC], F32, tag="lnv")
            r = pool_st.tile([80, NWC], F32, tag="r")
            rb = pool_st.tile([80, NWC], BF16, tag="rb")
            nc.scalar.activation(lnv, var, AF.Ln, bias=eps_sb[0:80],
                                 scale=1.0)
            nc.scalar.activation(r, lnv, AF.Exp, bias=0.0, scale=-0.5)
            nc.vector.tensor_copy(rb, r)
            return m, rb

        def emit_apply_transpose(x_wc, m, rb, fm, nm):
            """LN apply (h = (x - m) * r, bf16) in 4-window chunks, then
            per-window PE transposes into fm = [fm0, fm1] [128, 5120]."""
            for g in range(NBLKC):
                ws = slice(g * WBLK, (g + 1) * WBLK)
                h_bf = pool_ln.tile([80, WBLK, C], BF16, tag="h",
                                    name=f"h_{nm}_{g}")
                m_bc = m[:, ws, None].broadcast_to([80, WBLK, C])
                r_bc = rb[:, ws, None].broadcast_to([80, WBLK, C])
                if g % 2 == 0:
                    nc.gpsimd.tensor_tensor(h_bf, x_wc[:, ws], m_bc,
                                            ALU.subtract)
                    nc.vector.tensor_tensor(h_bf, h_bf, r_bc, ALU.mult)
                else:
                    nc.vector.tensor_tensor(h_bf, x_wc[:, ws], m_bc,
                                            ALU.subtract)
                    nc.gpsimd.tensor_tensor(h_bf, h_bf, r_bc, ALU.mult)
                for ch in range(2):
                    pt = psum_tr.tile([128, BLKTOK], BF16, tag="tr")
                    for wi in range(WBLK):
                        nc.tensor.matmul(
                            pt[:, wi * NT:(wi + 1) * NT],
                            h_bf[:, wi, ch * 128:(ch + 1) * 128],
                            id128[0:80, 0:80],
                            is_transpose=True)
                    dst = fm[ch][:, g * BLKTOK:(g + 1) * BLKTOK]
                    if (g + ch) % 2 == 0:
                        nc.scalar.activation(dst, pt, AF.Copy)
                    else:
                        nc.vector.tensor_copy(dst, pt)

        def emit_chunk(b, half):
            # ---- load x window-gathered (half = 32 windows: hh 4*half..) ----
            hh0 = half * (NWC // GW_W)
            x_wc = pool_x.tile([80, NWC, C], F32, tag="x",
                               name=f"x_{b}_{half}")
            x_wc4 = x_wc.rearrange("p (hh ww) c -> p hh ww c", hh=NWC // GW_W)
            for gh in range(GH):
                nc.sync.dma_start(
                    out=x_wc4[gh * GW:(gh + 1) * GW],
                    in_=x_g[b, gh][:, hh0:hh0 + NWC // GW_W])

            if KSTAGE < 2:
                emit_store(b, hh0, x_wc4)
                return

            # ---- LN1 + transpose to feature-major ----
            m1, rb1 = emit_ln(x_wc)
            hfm = [pool_fm.tile([128, NTOKC], BF16, tag="hfm", name=f"hfm{b}_{half}_{i}")
                   for i in range(2)]
            emit_apply_transpose(x_wc, m1, rb1, hfm, f"b{b}_{half}ln1")

            # ---- QKV: q, k (feature-major) ----
            # qk[0:2] = q tiles (4 heads each), qk[2:4] = k tiles
            qk = [pool_qk.tile([128, NTOKC], BF16, tag=f"qk{i}", name=f"qk{b}_{half}_{i}")
                  for i in range(4)]
            for g in range(NBLKC):
                sl = slice(g * BLKTOK, (g + 1) * BLKTOK)
                for mc in range(4):
                    pq = psum_big.tile([128, BLKTOK], F32, tag="big")
                    for kc in range(2):
                        nc.tensor.matmul(
                            pq, wqk_sb[kc][:, mc * 128:(mc + 1) * 128],
                            hfm[kc][:, sl],
                            start=(kc == 0), stop=(kc == 1))
                    if mc < 2:
                        nc.vector.tensor_copy(qk[mc][:, sl], pq)
                    else:
                        nc.scalar.activation(qk[mc][:, sl], pq, AF.Copy)

            if KSTAGE < 3:
                dummy = pool_ot.tile([80, C], BF16, tag="otm",
                                     name=f"dmy{b}_{half}")
                nc.vector.tensor_copy(dummy[0:64, 0:128],
                                      qk[0][0:64, 0:128])
                nc.vector.tensor_copy(dummy[0:64, 128:256],
                                      hfm[0][0:64, 0:128])
                emit_store(b, hh0, x_wc4)
                return

            # ---- attention + flipped proj + residual1 ----
            # NB: all matmuls writing one PSUM tile must share tile_position,
            # so S' groups by head class c = h % 4 (heads {c, c+4}) across a
            # window triple: 6 window-heads per tile, one position (32c, 0).
            ofm = [None, None]
            otp = [None, None]
            NWG = 3  # windows per S' group
            for w0 in range(0, NWC, NWG):
                nw = min(NWG, NWC - w0)
                egs = []
                for c in range(4):
                    ps = psum_sm.tile([80, 160 * NWG], F32, tag="sm",
                                      name=f"ps_{b}_{half}_{w0}_{c}")
                    for j in range(nw):
                        for hh in range(2):
                            h = c + 4 * hh
                            i = 2 * j + hh
                            ts = slice((w0 + j) * NT, (w0 + j + 1) * NT)
                            hs = slice(32 * c, 32 * c + 32)
                            nc.tensor.matmul(
                                ps[:, i * 80:(i + 1) * 80],
                                qk[2 + h // 4][hs, ts], qk[h // 4][hs, ts],
                                tile_position=(32 * c, 0))
                    eg = pool_e.tile([80, 160 * NWG], BF16, tag="e",
                                     name=f"eg_{b}_{half}_{w0}_{c}")
                    nc.scalar.activation(eg[:, :160 * nw], ps[:, :160 * nw],
                                         AF.Exp, bias=0.0, scale=SCALE)
                    egs.append(eg)
                if KATTN < 1:
                    continue

                for w in range(w0, w0 + nw):
                    # v for this window (flipped matmul, + ones column)
                    v33 = pool_v.tile([80, HEADS, 33], BF16, tag="v33",
                                      name=f"v33_{b}_{half}_{w}")
                    nc.gpsimd.memset(v33[:, :, 32], 1.0)
                    pv = psum_sm.tile([80, 256], F32, tag="sm")
                    for kc in range(2):
                        nc.tensor.matmul(
                            pv, hfm[kc][:, w * NT:(w + 1) * NT], wv_sb[kc],
                            start=(kc == 0), stop=(kc == 1))
                    dstv = v33[:, :, 0:32]
                    srcv = pv.rearrange("p (h d) -> p h d", h=HEADS)
                    if w % 2 == 0:
                        nc.vector.tensor_copy(dstv, srcv)
                    else:
                        nc.scalar.activation(dstv, srcv, AF.Copy)
                    po = psum_sm.tile([80, HEADS * 33], F32, tag="sm")
                    for h in range(HEADS):
                        c, hh = h % 4, h // 4
                        i = 2 * (w - w0) + hh
                        nc.tensor.matmul(po[:, h * 33:(h + 1) * 33],
                                         egs[c][:, i * 80:(i + 1) * 80],
                                         v33[:, h, :])
                    pov = po.rearrange("p (h d) -> p h d", h=HEADS)
                    r8 = pool_st.tile([80, HEADS], F32, tag="r8")
                    nc.vector.reciprocal(r8, pov[:, :, 32])
                    otm = pool_ot.tile([80, C], BF16, tag="otm")
                    nc.vector.tensor_tensor(
                        otm.rearrange("p (h d) -> p h d", h=HEADS),
                        pov[:, :, 0:32],
                        r8[:, :, None].broadcast_to([80, HEADS, 32]),
                        ALU.mult)
                    if KATTN < 2:
                        continue
                    # transpose O into a per-4-window psum group; evict and
                    # run proj + residual once the group is complete
                    wi = w % WBLK
                    if wi == 0:
                        ofm[0] = pool_of.tile([128, BLKTOK], BF16,
                                              tag="of0", name=f"of0_{b}_{half}_{w}")
                        ofm[1] = pool_of.tile([128, BLKTOK], BF16,
                                              tag="of1", name=f"of1_{b}_{half}_{w}")
                        otp[0] = psum_tr.tile([128, 2, BLKTOK], BF16, tag="tr",
                                              name=f"otp_{b}_{half}_{w}")
                    for ch in range(2):
                        nc.tensor.matmul(otp[0][:, ch, wi * NT:(wi + 1) * NT],
                                         otm[:, ch * 128:(ch + 1) * 128],
                                         id128[0:80, 0:80],
                                         is_transpose=True)
                    if wi < WBLK - 1:
                        continue
                    for ch in range(2):
                        nc.scalar.activation(ofm[ch], otp[0][:, ch], AF.Copy)
                    if KATTN < 3:
                        continue
                    for wj in range(WBLK):
                        wq = w - (WBLK - 1) + wj
                        pp = psum_sm.tile([80, 256], F32, tag="sm",
                                          name=f"pp_{b}_{half}_{wq}")
                        for kc in range(2):
                            nc.tensor.matmul(
                                pp, ofm[kc][:, wj * NT:(wj + 1) * NT],
                                wp_sb[kc], start=(kc == 0), stop=(kc == 1))
                        nc.vector.tensor_tensor(x_wc[:, wq], x_wc[:, wq], pp,
                                                ALU.add)

            if KSTAGE < 4:
                emit_store(b, hh0, x_wc4)
                return

            # ---- LN2 + transpose ----
            m2, rb2 = emit_ln(x_wc)
            h2fm = [pool_fm.tile([128, NTOKC], BF16, tag="hfm",
                                 name=f"h2fm{b}_{half}_{i}") for i in range(2)]
            emit_apply_transpose(x_wc, m2, rb2, h2fm, f"b{b}_{half}ln2")

            # ---- MLP ----
            for g in range(NBLKC):
                sl = slice(g * BLKTOK, (g + 1) * BLKTOK)
                gsb = pool_g.tile([128, 8, BLKTOK], BF16, tag="g")
                for mc in range(8):
                    pf = psum_big.tile([128, BLKTOK], F32, tag="big")
                    for kc in range(2):
                        nc.tensor.matmul(
                            pf, wf1_sb[kc][:, mc * 128:(mc + 1) * 128],
                            h2fm[kc][:, sl],
                            start=(kc == 0), stop=(kc == 1))
                    nc.scalar.activation(gsb[:, mc], pf, AF.Gelu)
                f2 = [pool_f2.tile([128, BLKTOK], BF16, tag=f"f2{mc}",
                                name=f"f2_{b}_{half}_{g}_{mc}") for mc in range(2)]
                for mc in range(2):
                    pa = psum_acc.tile([128, BLKTOK], F32, tag="acc")
                    for kc in range(8):
                        nc.tensor.matmul(
                            pa, wf2_sb[kc][:, mc * 128:(mc + 1) * 128],
                            gsb[:, kc],
                            start=(kc == 0), stop=(kc == 7))
                    nc.vector.tensor_copy(f2[mc], pa)
                # transpose back + residual2 in place
                for mc in range(2):
                    pt = psum_tr.tile([80, WBLK * 128], BF16, tag="tr")
                    for wi in range(WBLK):
                        nc.tensor.matmul(
                            pt[:, wi * 128:(wi + 1) * 128],
                            f2[mc][:, wi * NT:(wi + 1) * NT],
                            id128, is_transpose=True)
                    xsl = x_wc[:, g * WBLK:(g + 1) * WBLK,
                               mc * 128:(mc + 1) * 128]
                    nc.vector.tensor_tensor(
                        xsl, xsl, pt.rearrange("p (w c) -> p w c", w=WBLK),
                        ALU.add)

            # ---- store ----
            emit_store(b, hh0, x_wc4)

        for b in range(B_LOC):
            for half in range(2):
                emit_chunk(b, half)

        for p in reversed((consts, pool_x, pool_ln, pool_fm, pool_qk,
                           pool_v, pool_e, pool_ot, pool_of, pool_g, pool_f2,
                           pool_st, psum_big, psum_acc, psum_sm, psum_tr)):
            p.release()

    nc.compile()
    return nc


_NC_CACHE = None


def _get_nc():
    global _NC_CACHE
    if _NC_CACHE is None:
        _NC_CACHE = build_nc()
    return _NC_CACHE


def _prep_weights(norm1_g, norm1_b, qkv_w, qkv_b, proj_w, proj_b, ls1_g,
                  norm2_g, norm2_b, fc1_w, fc1_b, fc2_w, fc2_b, ls2_g):
    """Host-side weight folding. Returns dict of device weight arrays.

    gamma folds into the following matmul's weights; beta/bias terms must
    be zero (true for this module's init) — asserted here.
    """
    qkv_w = np.asarray(qkv_w, np.float32)
    w_eff = np.asarray(norm1_g, np.float32)[:, None] * qkv_w
    b_eff = np.asarray(norm1_b, np.float32) @ qkv_w + np.asarray(qkv_b)
    f1_eff = np.asarray(norm2_g, np.float32)[:, None] * np.asarray(fc1_w)
    f1b_eff = np.asarray(norm2_b, np.float32) @ np.asarray(fc1_w) + fc1_b
    wp_eff = np.asarray(proj_w, np.float32) * np.asarray(ls1_g)[None, :]
    pb_eff = np.asarray(proj_b) * np.asarray(ls1_g)
    wf2_eff = np.asarray(fc2_w, np.float32) * np.asarray(ls2_g)[None, :]
    f2b_eff = np.asarray(fc2_b) * np.asarray(ls2_g)
    for nm, v in [("qkv_b", b_eff), ("fc1_b", f1b_eff), ("proj_b", pb_eff),
                  ("fc2_b", f2b_eff)]:
        assert np.allclose(np.asarray(v), 0.0, atol=1e-30), \
            f"nonzero {nm} not supported by this kernel build"
    return {
        "wqk": _bf16(w_eff[:, :512]).reshape(2, 128, 512),
        "wv": _bf16(w_eff[:, 512:768]).reshape(2, 128, 256),
        "wp": _bf16(wp_eff).reshape(2, 128, 256),
        "wf1": _bf16(f1_eff).reshape(2, 128, INNER),
        "wf2": _bf16(wf2_eff).reshape(8, 128, 256),
    }


def run_sharded(inputs, trace=False, trace_kwargs=None):
    """inputs: full-problem dict from setup_inputs(). Returns
    (out [B,H,W,C] f32, BassKernelResults)."""
    nc = _get_nc()
    x = np.asarray(inputs["x"], np.float32)
    wmap = _prep_weights(
        inputs["norm1_g"], inputs["norm1_b"], inputs["qkv_w"],
        inputs["qkv_b"], inputs["proj_w"], inputs["proj_b"], inputs["ls1_g"],
        inputs["norm2_g"], inputs["norm2_b"], inputs["fc1_w"],
        inputs["fc1_b"], inputs["fc2_w"], inputs["fc2_b"], inputs["ls2_g"])
    in_maps = []
    for c in range(NCORES):
        m = dict(wmap)
        m["x"] = np.ascontiguousarray(x[c * B_LOC:(c + 1) * B_LOC])
        in_maps.append(m)
    kw = {}
    if trace:
        kw["trace"] = True
        kw["trace_kwargs"] = trace_kwargs or {}
    res = bass_utils.run_bass_kernel_spmd(nc, in_maps,
                                          core_ids=list(range(NCORES)), **kw)
    out = np.concatenate([res.results[c]["out"] for c in range(NCORES)],
                         axis=0)
    return out, res


def kernel(**inputs) -> np.ndarray:
    out, _ = run_sharded(inputs)
    return out.astype(np.float32)


if __name__ == "__main__":
    nc = build_nc()
    print("built + compiled ok")


# revision 42
# speedup vs baseline: 1.0749x; 1.0749x over previous
"""Trainium2 Bass kernel for MaxViT-style grid-attention block.

Full module: x -> LN1 -> grid-partition attention (8 heads, 80-token
windows) -> layerscale residual -> LN2 -> MLP(256->1024 GELU ->256) ->
layerscale residual.

Sharding: data-parallel over batch B=16 across 8 cores (2 batch elems
per core); weights replicated.

Per-core dataflow (per batch element, 64 windows x 80 tokens):
  - x loaded window-gathered into "window-column" layout [80, 64, 256]
    (partition = token-in-window, free = (window, channel)).
  - LN1 stats via bn_stats; apply via two broadcast tensor_tensor ops
    (gamma/beta folded into weights/biases on host).
  - h transposed per-window to feature-major via PE transposes.
  - QKV: q,k via standard fm matmuls; v via flipped matmuls directly
    into token-major [80, 256] per window (+ ones column for the
    softmax denominator trick).
  - S' = k^T q per (window, head) -> [keys, q] in PSUM; exp on ACT
    (scale folded); PV with E as stationary and [v | 1] as moving gives
    O_tm [80q, 33] with the denominator in column 32. Normalize with
    per-partition reciprocal on eviction.
  - proj flipped (stationary = O_fm) to produce token-major proj out,
    residual-added in place into x (layerscale folded into proj_w).
  - LN2 same as LN1; MLP feature-major; fc2 output transposed back and
    residual-added in place (layerscale folded into fc2_w).
"""

import os
import sys

sys.path.insert(0, "/opt/trn_rl_repo")

KSTAGE = int(os.environ.get("KSTAGE", "4"))
KATTN = int(os.environ.get("KATTN", "3"))

import numpy as np
import ml_dtypes

import concourse.bass as bass
import concourse.bacc as bacc
import concourse.tile as tile
from concourse import mybir
from concourse import bass_utils
from concourse.masks import make_identity

F32 = mybir.dt.float32
BF16 = mybir.dt.bfloat16
AF = mybir.ActivationFunctionType
ALU = mybir.AluOpType

# Problem constants (hardcoded per contract)
B, H, W, C = 16, 64, 80, 256
GH, GW = 8, 10
HEADS, DH = 8, 32
INNER = 1024
SCALE = DH**-0.5
EPS = 1e-5

NCORES = 8
B_LOC = B // NCORES          # 2 batch elems per core
NWIN = (H // GH) * (W // GW)  # 64 windows per batch elem
NT = GH * GW                  # 80 tokens per window
NTOK = NWIN * NT              # 5120 tokens per batch elem
WBLK = 4                      # windows per token-block (320 tokens)
NBLK = NWIN // WBLK           # 16 token-blocks
BLKTOK = WBLK * NT            # 320


def _bf16(a):
    return np.asarray(a, np.float32).astype(ml_dtypes.bfloat16)


def build_nc():
    nc = bacc.Bacc("TRN2", target_bir_lowering=False, debug=False,
                   enable_asserts=False)

    # ---- DRAM I/O (per-core shapes) ----
    x_d = nc.dram_tensor("x", [B_LOC, H, W, C], F32, kind="ExternalInput")
    out_d = nc.dram_tensor("out", [B_LOC, H, W, C], F32, kind="ExternalOutput")
    wqk_d = nc.dram_tensor("wqk", [2, 128, 512], BF16, kind="ExternalInput")
    wv_d = nc.dram_tensor("wv", [2, 128, 256], BF16, kind="ExternalInput")
    wp_d = nc.dram_tensor("wp", [2, 128, 256], BF16, kind="ExternalInput")
    wf1_d = nc.dram_tensor("wf1", [2, 128, INNER], BF16, kind="ExternalInput")
    wf2_d = nc.dram_tensor("wf2", [8, 128, 256], BF16, kind="ExternalInput")

    # window-gathered views of x / out:
    # [b, (gh hh), (gw ww), c] -> [b, gh, gw, (hh ww), c]
    x_g = x_d.ap().rearrange("b (gh hh) (gw ww) c -> b gh gw hh ww c",
                             gh=GH, gw=GW)
    out_g = out_d.ap().rearrange("b (gh hh) (gw ww) c -> b gh gw hh ww c",
                                 gh=GH, gw=GW)

    with tile.TileContext(nc) as tc:
        consts = tc.alloc_tile_pool(name="consts", bufs=1)
        pool_x = tc.alloc_tile_pool(name="x", bufs=2)
        pool_ln = tc.alloc_tile_pool(name="ln", bufs=4)
        pool_fm = tc.alloc_tile_pool(name="fm", bufs=6)
        pool_qk = tc.alloc_tile_pool(name="qk", bufs=2)
        pool_v = tc.alloc_tile_pool(name="v", bufs=5)
        pool_e = tc.alloc_tile_pool(name="e", bufs=12)
        pool_ot = tc.alloc_tile_pool(name="ot", bufs=12)
        pool_of = tc.alloc_tile_pool(name="of", bufs=4)
        pool_g = tc.alloc_tile_pool(name="g", bufs=3)
        pool_f2 = tc.alloc_tile_pool(name="f2", bufs=4)
        pool_st = tc.alloc_tile_pool(name="st", bufs=3)
        psum_big = tc.alloc_tile_pool(name="pbig", bufs=2, space="PSUM")
        psum_acc = tc.alloc_tile_pool(name="pacc", bufs=1, space="PSUM")
        psum_sm = tc.alloc_tile_pool(name="psm", bufs=4, space="PSUM")
        psum_tr = tc.alloc_tile_pool(name="ptr", bufs=1, space="PSUM")

        # ---- constants ----
        id128 = consts.tile([128, 128], BF16)
        make_identity(nc, id128)
        eps_sb = consts.tile([128, 1], F32)
        nc.gpsimd.memset(eps_sb, EPS)

        def load_w(dram_ap, n, shape, nm):
            ts = []
            for i in range(n):
                t = consts.tile(shape, BF16, name=f"{nm}{i}")
                nc.sync.dma_start(out=t, in_=dram_ap[i])
                ts.append(t)
            return ts

        wqk_sb = load_w(wqk_d.ap(), 2, [128, 512], "wqk")
        wv_sb = load_w(wv_d.ap(), 2, [128, 256], "wv")
        wp_sb = load_w(wp_d.ap(), 2, [128, 256], "wp")
        wf1_sb = load_w(wf1_d.ap(), 2, [128, INNER], "wf1")
        wf2_sb = load_w(wf2_d.ap(), 8, [128, 256], "wf2")

        NWC = 32            # windows per chunk (half a batch elem)
        NTOKC = NWC * NT    # 2560
        NBLKC = NWC // WBLK  # 8
        GW_W = GH           # hh count per half = NWC // GW_W = 4

        def emit_store(b, hh0, x_wc4):
            for gh in range(GH):
                nc.sync.dma_start(
                    out=out_g[b, gh][:, hh0:hh0 + NWC // GW_W],
                    in_=x_wc4[gh * GW:(gh + 1) * GW])

        def emit_ln(x_wc, on_act=False):
            """x_wc [80, 64, 256] f32 -> per-token (mean, 1/std as bf16);
            gamma/beta folded into downstream weights. Stats on DVE
            (bn_stats) or ACT (Square/Identity with accum_out)."""
            m = pool_st.tile([80, NWC], F32, tag="m")
            var = pool_st.tile([80, NWC], F32, tag="var")
            t0 = pool_st.tile([80, NWC], F32, tag="t0")
            t1 = pool_st.tile([80, NWC], F32, tag="t1")
            if on_act:
                sums = pool_st.tile([80, NWC], F32, tag="sums")
                sumsq = pool_st.tile([80, NWC], F32, tag="sumsq")
                for w0 in range(NWC):
                    scr = pool_ln.tile([80, C], BF16, tag="scr",
                                       name=f"scr_{w0}")
                    nc.scalar.activation(scr, x_wc[:, w0], AF.Identity,
                                         accum_out=sums[:, w0:w0 + 1])
                    nc.scalar.activation(scr, x_wc[:, w0], AF.Square,
                                         accum_out=sumsq[:, w0:w0 + 1])
                # mean = sum/C ; var = sumsq/C - mean^2
                nc.vector.tensor_scalar(m, sums, 1.0 / C, None, ALU.mult)
                nc.vector.tensor_tensor(t1, m, m, ALU.mult)
                nc.vector.tensor_scalar(t0, sumsq, 1.0 / C, None, ALU.mult)
                nc.vector.tensor_tensor(var, t0, t1, ALU.subtract)
            else:
                st6 = pool_st.tile([80, NWC, 6], F32, tag="st6")
                for w0 in range(NWC):
                    nc.vector.bn_stats(st6[:, w0], x_wc[:, w0])
                # mean = (m_even + m_odd) / 2
                nc.vector.tensor_tensor(t0, st6[:, :, 1], st6[:, :, 4],
                                        ALU.add)
                nc.vector.tensor_scalar(m, t0, 0.5, None, ALU.mult)
                # var = (cv_e + cv_o)/256 + ((m_e - m_o)/2)^2
                nc.vector.tensor_tensor(t0, st6[:, :, 2], st6[:, :, 5],
                                        ALU.add)
                nc.vector.tensor_tensor(t1, st6[:, :, 1], st6[:, :, 4],
                                        ALU.subtract)
                nc.vector.tensor_tensor(t1, t1, t1, ALU.mult)
                nc.vector.tensor_scalar(t0, t0, 1.0 / C, None, ALU.mult)
                nc.vector.tensor_scalar(t1, t1, 0.25, None, ALU.mult)
                nc.vector.tensor_tensor(var, t0, t1, ALU.add)
            # r = rsqrt(var + eps) = exp(-0.5 * ln(var + eps))
            lnv = pool_st.tile([80, NWC], F32, tag="lnv")
            r = pool_st.tile([80, NWC], F32, tag="r")
            rb = pool_st.tile([80, NWC], BF16, tag="rb")
            nc.scalar.activation(lnv, var, AF.Ln, bias=eps_sb[0:80],
                                 scale=1.0)
            nc.scalar.activation(r, lnv, AF.Exp, bias=0.0, scale=-0.5)
            nc.vector.tensor_copy(rb, r)
            return m, rb, r, None

        def emit_apply_transpose(x_wc, lnstats, fm, nm):
            """LN apply (h = (x - m) * r, bf16) in 4-window chunks, then
            per-window PE transposes into fm = [fm0, fm1] [128, 5120].
            Apply rotates across gpsimd/DVE TT pairs and fused per-window
            ACT ops (func(scale*x + bias) with per-partition scale/bias)."""
            m, rb, r, negmr = lnstats
            for g in range(NBLKC):
                ws = slice(g * WBLK, (g + 1) * WBLK)
                h_bf = pool_ln.tile([80, WBLK, C], BF16, tag="h",
                                    name=f"h_{nm}_{g}")
                m_bc = m[:, ws, None].broadcast_to([80, WBLK, C])
                r_bc = rb[:, ws, None].broadcast_to([80, WBLK, C])
                if g % 2 == 0:
                    nc.gpsimd.tensor_tensor(h_bf, x_wc[:, ws], m_bc,
                                            ALU.subtract)
                    nc.vector.tensor_tensor(h_bf, h_bf, r_bc, ALU.mult)
                else:
                    nc.vector.tensor_tensor(h_bf, x_wc[:, ws], m_bc,
                                            ALU.subtract)
                    nc.gpsimd.tensor_tensor(h_bf, h_bf, r_bc, ALU.mult)
                for ch in range(2):
                    pt = psum_tr.tile([128, BLKTOK], BF16, tag="tr")
                    for wi in range(WBLK):
                        nc.tensor.matmul(
                            pt[:, wi * NT:(wi + 1) * NT],
                            h_bf[:, wi, ch * 128:(ch + 1) * 128],
                            id128[0:80, 0:80],
                            is_transpose=True)
                    dst = fm[ch][:, g * BLKTOK:(g + 1) * BLKTOK]
                    if (g + ch) % 2 == 0:
                        nc.scalar.activation(dst, pt, AF.Copy)
                    else:
                        nc.vector.tensor_copy(dst, pt)

        def emit_chunk(b, half):
            # ---- load x window-gathered (half = 32 windows: hh 4*half..) ----
            hh0 = half * (NWC // GW_W)
            x_wc = pool_x.tile([80, NWC, C], F32, tag="x",
                               name=f"x_{b}_{half}")
            x_wc4 = x_wc.rearrange("p (hh ww) c -> p hh ww c", hh=NWC // GW_W)
            for gh in range(GH):
                nc.sync.dma_start(
                    out=x_wc4[gh * GW:(gh + 1) * GW],
                    in_=x_g[b, gh][:, hh0:hh0 + NWC // GW_W])

            if KSTAGE < 2:
                emit_store(b, hh0, x_wc4)
                return

            # ---- LN1 + transpose to feature-major ----
            ln1 = emit_ln(x_wc)
            hfm = [pool_fm.tile([128, NTOKC], BF16, tag="hfm", name=f"hfm{b}_{half}_{i}")
                   for i in range(2)]
            emit_apply_transpose(x_wc, ln1, hfm, f"b{b}_{half}ln1")

            # ---- QKV: q, k (feature-major) ----
            # qk[0:2] = q tiles (4 heads each), qk[2:4] = k tiles
            qk = [pool_qk.tile([128, NTOKC], BF16, tag=f"qk{i}", name=f"qk{b}_{half}_{i}")
                  for i in range(4)]
            for g in range(NBLKC):
                sl = slice(g * BLKTOK, (g + 1) * BLKTOK)
                for mc in range(4):
                    pq = psum_big.tile([128, BLKTOK], F32, tag="big")
                    for kc in range(2):
                        nc.tensor.matmul(
                            pq, wqk_sb[kc][:, mc * 128:(mc + 1) * 128],
                            hfm[kc][:, sl],
                            start=(kc == 0), stop=(kc == 1))
                    if mc < 2:
                        nc.vector.tensor_copy(qk[mc][:, sl], pq)
                    else:
                        nc.scalar.activation(qk[mc][:, sl], pq, AF.Copy)

            if KSTAGE < 3:
                dummy = pool_ot.tile([80, C], BF16, tag="otm",
                                     name=f"dmy{b}_{half}")
                nc.vector.tensor_copy(dummy[0:64, 0:128],
                                      qk[0][0:64, 0:128])
                nc.vector.tensor_copy(dummy[0:64, 128:256],
                                      hfm[0][0:64, 0:128])
                emit_store(b, hh0, x_wc4)
                return

            # ---- attention + flipped proj + residual1 ----
            # NB: all matmuls writing one PSUM tile must share tile_position,
            # so S' groups by head class c = h % 4 (heads {c, c+4}) across a
            # window triple: 6 window-heads per tile, one position (32c, 0).
            # v (flipped matmuls, + ones column) in window-pairs, emitted
            # on demand just ahead of each attention group (pool-depth bound)
            v33t = {}

            def emit_v_pair(vp):
                wp = vp * 2
                v33 = pool_v.tile([80, 2, HEADS, 33], BF16, tag="v33",
                                  name=f"v33_{b}_{half}_{wp}")
                nc.gpsimd.memset(v33[:, :, :, 32], 1.0)
                pv = psum_sm.tile([80, 2, 256], F32, tag="sm",
                                  name=f"pv_{b}_{half}_{wp}")
                for u in range(2):
                    for kc in range(2):
                        nc.tensor.matmul(
                            pv[:, u],
                            hfm[kc][:, (wp + u) * NT:(wp + u + 1) * NT],
                            wv_sb[kc], start=(kc == 0), stop=(kc == 1))
                dstv = v33[:, :, :, 0:32]
                srcv = pv.rearrange("p u (h d) -> p u h d", h=HEADS)
                if vp % 2 == 0:
                    nc.vector.tensor_copy(dstv, srcv)
                else:
                    nc.scalar.activation(dstv, srcv, AF.Copy)
                v33t[vp] = v33

            ofm = [None, None]
            otp = [None, None]
            NWG = 3  # windows per S' group
            next_vp = 0
            for w0 in range(0, NWC, NWG):
                nw = min(NWG, NWC - w0)
                while next_vp * 2 < w0 + nw:
                    emit_v_pair(next_vp)
                    next_vp += 1
                egs = []
                for c in range(4):
                    ps = psum_sm.tile([80, 160 * NWG], F32, tag="sm",
                                      name=f"ps_{b}_{half}_{w0}_{c}")
                    for j in range(nw):
                        for hh in range(2):
                            h = c + 4 * hh
                            i = 2 * j + hh
                            ts = slice((w0 + j) * NT, (w0 + j + 1) * NT)
                            hs = slice(32 * c, 32 * c + 32)
                            nc.tensor.matmul(
                                ps[:, i * 80:(i + 1) * 80],
                                qk[2 + h // 4][hs, ts], qk[h // 4][hs, ts],
                                tile_position=(32 * c, 0))
                    eg = pool_e.tile([80, 160 * NWG], BF16, tag="e",
                                     name=f"eg_{b}_{half}_{w0}_{c}")
                    nc.scalar.activation(eg[:, :160 * nw], ps[:, :160 * nw],
                                         AF.Exp, bias=0.0, scale=SCALE)
                    egs.append(eg)
                if KATTN < 1:
                    continue

                for w in range(w0, w0 + nw):
                    po = psum_sm.tile([80, HEADS * 33], F32, tag="sm")
                    for h in range(HEADS):
                        c, hh = h % 4, h // 4
                        i = 2 * (w - w0) + hh
                        nc.tensor.matmul(po[:, h * 33:(h + 1) * 33],
                                         egs[c][:, i * 80:(i + 1) * 80],
                                         v33t[w // 2][:, w % 2, h, :])
                    pov = po.rearrange("p (h d) -> p h d", h=HEADS)
                    r8 = pool_st.tile([80, HEADS], F32, tag="r8")
                    nc.vector.reciprocal(r8, pov[:, :, 32])
                    otm = pool_ot.tile([80, C], BF16, tag="otm")
                    nc.vector.tensor_tensor(
                        otm.rearrange("p (h d) -> p h d", h=HEADS),
                        pov[:, :, 0:32],
                        r8[:, :, None].broadcast_to([80, HEADS, 32]),
                        ALU.mult)
                    if KATTN < 2:
                        continue
                    # transpose O into a per-4-window psum group; evict and
                    # run proj + residual once the group is complete
                    wi = w % WBLK
                    if wi == 0:
                        ofm[0] = pool_of.tile([128, BLKTOK], BF16,
                                              tag="of0", name=f"of0_{b}_{half}_{w}")
                        ofm[1] = pool_of.tile([128, BLKTOK], BF16,
                                              tag="of1", name=f"of1_{b}_{half}_{w}")
                        otp[0] = psum_tr.tile([128, 2, BLKTOK], BF16, tag="tr",
                                              name=f"otp_{b}_{half}_{w}")
                    for ch in range(2):
                        nc.tensor.matmul(otp[0][:, ch, wi * NT:(wi + 1) * NT],
                                         otm[:, ch * 128:(ch + 1) * 128],
                                         id128[0:80, 0:80],
                                         is_transpose=True)
                    if wi < WBLK - 1:
                        continue
                    for ch in range(2):
                        nc.scalar.activation(ofm[ch], otp[0][:, ch], AF.Copy)
                    if KATTN < 3:
                        continue
                    for wj in range(0, WBLK, 2):
                        wq = w - (WBLK - 1) + wj
                        pp = psum_sm.tile([80, 2, 256], F32, tag="sm",
                                          name=f"pp_{b}_{half}_{wq}")
                        for u in range(2):
                            for kc in range(2):
                                nc.tensor.matmul(
                                    pp[:, u],
                                    ofm[kc][:, (wj + u) * NT:
                                            (wj + u + 1) * NT],
                                    wp_sb[kc], start=(kc == 0),
                                    stop=(kc == 1))
                        nc.vector.tensor_tensor(x_wc[:, wq:wq + 2],
                                                x_wc[:, wq:wq + 2], pp,
                                                ALU.add)

            if KSTAGE < 4:
                emit_store(b, hh0, x_wc4)
                return

            # ---- LN2 + transpose ----
            ln2 = emit_ln(x_wc)
            h2fm = [pool_fm.tile([128, NTOKC], BF16, tag="hfm",
                                 name=f"h2fm{b}_{half}_{i}") for i in range(2)]
            emit_apply_transpose(x_wc, ln2, h2fm, f"b{b}_{half}ln2")

            # ---- MLP ----
            for g in range(NBLKC):
                sl = slice(g * BLKTOK, (g + 1) * BLKTOK)
                gsb = pool_g.tile([128, 8, BLKTOK], BF16, tag="g")
                for mc in range(8):
                    pf = psum_big.tile([128, BLKTOK], F32, tag="big")
                    for kc in range(2):
                        nc.tensor.matmul(
                            pf, wf1_sb[kc][:, mc * 128:(mc + 1) * 128],
                            h2fm[kc][:, sl],
                            start=(kc == 0), stop=(kc == 1))
                    nc.scalar.activation(gsb[:, mc], pf, AF.Gelu)
                f2 = [pool_f2.tile([128, BLKTOK], BF16, tag=f"f2{mc}",
                                name=f"f2_{b}_{half}_{g}_{mc}") for mc in range(2)]
                for mc in range(2):
                    pa = psum_acc.tile([128, BLKTOK], F32, tag="acc")
                    for kc in range(8):
                        nc.tensor.matmul(
                            pa, wf2_sb[kc][:, mc * 128:(mc + 1) * 128],
                            gsb[:, kc],
                            start=(kc == 0), stop=(kc == 7))
                    nc.vector.tensor_copy(f2[mc], pa)
                # transpose back + residual2 in place
                for mc in range(2):
                    pt = psum_tr.tile([80, WBLK * 128], BF16, tag="tr")
                    for wi in range(WBLK):
                        nc.tensor.matmul(
                            pt[:, wi * 128:(wi + 1) * 128],
                            f2[mc][:, wi * NT:(wi + 1) * NT],
                            id128, is_transpose=True)
                    xsl = x_wc[:, g * WBLK:(g + 1) * WBLK,
                               mc * 128:(mc + 1) * 128]
                    nc.vector.tensor_tensor(
                        xsl, xsl, pt.rearrange("p (w c) -> p w c", w=WBLK),
                        ALU.add)

            # ---- store ----
            emit_store(b, hh0, x_wc4)

        for b in range(B_LOC):
            for half in range(2):
                emit_chunk(b, half)

        for p in reversed((consts, pool_x, pool_ln, pool_fm, pool_qk,
                           pool_v, pool_e, pool_ot, pool_of, pool_g, pool_f2,
                           pool_st, psum_big, psum_acc, psum_sm, psum_tr)):
            p.release()

    nc.compile()
    return nc


_NC_CACHE = None


def _get_nc():
    global _NC_CACHE
    if _NC_CACHE is None:
        _NC_CACHE = build_nc()
    return _NC_CACHE


def _prep_weights(norm1_g, norm1_b, qkv_w, qkv_b, proj_w, proj_b, ls1_g,
                  norm2_g, norm2_b, fc1_w, fc1_b, fc2_w, fc2_b, ls2_g):
    """Host-side weight folding. Returns dict of device weight arrays.

    gamma folds into the following matmul's weights; beta/bias terms must
    be zero (true for this module's init) — asserted here.
    """
    qkv_w = np.asarray(qkv_w, np.float32)
    w_eff = np.asarray(norm1_g, np.float32)[:, None] * qkv_w
    b_eff = np.asarray(norm1_b, np.float32) @ qkv_w + np.asarray(qkv_b)
    f1_eff = np.asarray(norm2_g, np.float32)[:, None] * np.asarray(fc1_w)
    f1b_eff = np.asarray(norm2_b, np.float32) @ np.asarray(fc1_w) + fc1_b
    wp_eff = np.asarray(proj_w, np.float32) * np.asarray(ls1_g)[None, :]
    pb_eff = np.asarray(proj_b) * np.asarray(ls1_g)
    wf2_eff = np.asarray(fc2_w, np.float32) * np.asarray(ls2_g)[None, :]
    f2b_eff = np.asarray(fc2_b) * np.asarray(ls2_g)
    for nm, v in [("qkv_b", b_eff), ("fc1_b", f1b_eff), ("proj_b", pb_eff),
                  ("fc2_b", f2b_eff)]:
        assert np.allclose(np.asarray(v), 0.0, atol=1e-30), \
            f"nonzero {nm} not supported by this kernel build"
    return {
        "wqk": _bf16(w_eff[:, :512]).reshape(2, 128, 512),
        "wv": _bf16(w_eff[:, 512:768]).reshape(2, 128, 256),
        "wp": _bf16(wp_eff).reshape(2, 128, 256),
        "wf1": _bf16(f1_eff).reshape(2, 128, INNER),
        "wf2": _bf16(wf2_eff).reshape(8, 128, 256),
    }


def run_sharded(inputs, trace=False, trace_kwargs=None):
    """inputs: full-problem dict from setup_inputs(). Returns
    (out [B,H,W,C] f32, BassKernelResults)."""
    nc = _get_nc()
    x = np.asarray(inputs["x"], np.float32)
    wmap = _prep_weights(
        inputs["norm1_g"], inputs["norm1_b"], inputs["qkv_w"],
        inputs["qkv_b"], inputs["proj_w"], inputs["proj_b"], inputs["ls1_g"],
        inputs["norm2_g"], inputs["norm2_b"], inputs["fc1_w"],
        inputs["fc1_b"], inputs["fc2_w"], inputs["fc2_b"], inputs["ls2_g"])
    in_maps = []
    for c in range(NCORES):
        m = dict(wmap)
        m["x"] = np.ascontiguousarray(x[c * B_LOC:(c + 1) * B_LOC])
        in_maps.append(m)
    kw = {}
    if trace:
        kw["trace"] = True
        kw["trace_kwargs"] = trace_kwargs or {}
    res = bass_utils.run_bass_kernel_spmd(nc, in_maps,
                                          core_ids=list(range(NCORES)), **kw)
    out = np.concatenate([res.results[c]["out"] for c in range(NCORES)],
                         axis=0)
    return out, res


def kernel(**inputs) -> np.ndarray:
    out, _ = run_sharded(inputs)
    return out.astype(np.float32)


if __name__ == "__main__":
    nc = build_nc()
    print("built + compiled ok")


# revision 46
# speedup vs baseline: 1.1942x; 1.1110x over previous
"""Trainium2 Bass kernel for MaxViT-style grid-attention block.

Full module: x -> LN1 -> grid-partition attention (8 heads, 80-token
windows) -> layerscale residual -> LN2 -> MLP(256->1024 GELU ->256) ->
layerscale residual.

Sharding: data-parallel over batch B=16 across 8 cores (2 batch elems
per core); weights replicated.

Per-core dataflow (per batch element, 64 windows x 80 tokens):
  - x loaded window-gathered into "window-column" layout [80, 64, 256]
    (partition = token-in-window, free = (window, channel)).
  - LN1 stats via bn_stats; apply via two broadcast tensor_tensor ops
    (gamma/beta folded into weights/biases on host).
  - h transposed per-window to feature-major via PE transposes.
  - QKV: q,k via standard fm matmuls; v via flipped matmuls directly
    into token-major [80, 256] per window (+ ones column for the
    softmax denominator trick).
  - S' = k^T q per (window, head) -> [keys, q] in PSUM; exp on ACT
    (scale folded); PV with E as stationary and [v | 1] as moving gives
    O_tm [80q, 33] with the denominator in column 32. Normalize with
    per-partition reciprocal on eviction.
  - proj flipped (stationary = O_fm) to produce token-major proj out,
    residual-added in place into x (layerscale folded into proj_w).
  - LN2 same as LN1; MLP feature-major; fc2 output transposed back and
    residual-added in place (layerscale folded into fc2_w).
"""

import os
import sys

sys.path.insert(0, "/opt/trn_rl_repo")

KSTAGE = int(os.environ.get("KSTAGE", "4"))
KATTN = int(os.environ.get("KATTN", "3"))

import numpy as np
import ml_dtypes

import concourse.bass as bass
import concourse.bacc as bacc
import concourse.tile as tile
from concourse import mybir
from concourse import bass_utils
from concourse.masks import make_identity

F32 = mybir.dt.float32
BF16 = mybir.dt.bfloat16
AF = mybir.ActivationFunctionType
ALU = mybir.AluOpType

# Problem constants (hardcoded per contract)
B, H, W, C = 16, 64, 80, 256
GH, GW = 8, 10
HEADS, DH = 8, 32
INNER = 1024
SCALE = DH**-0.5
EPS = 1e-5

NCORES = 8
B_LOC = B // NCORES          # 2 batch elems per core
NWIN = (H // GH) * (W // GW)  # 64 windows per batch elem
NT = GH * GW                  # 80 tokens per window
NTOK = NWIN * NT              # 5120 tokens per batch elem
WBLK = 4                      # windows per token-block (320 tokens)
NBLK = NWIN // WBLK           # 16 token-blocks
BLKTOK = WBLK * NT            # 320


def _bf16(a):
    return np.asarray(a, np.float32).astype(ml_dtypes.bfloat16)


def build_nc():
    nc = bacc.Bacc("TRN2", target_bir_lowering=False, debug=False,
                   enable_asserts=False)

    # ---- DRAM I/O (per-core shapes) ----
    x_d = nc.dram_tensor("x", [B_LOC, H, W, C], F32, kind="ExternalInput")
    out_d = nc.dram_tensor("out", [B_LOC, H, W, C], F32, kind="ExternalOutput")
    wqk_d = nc.dram_tensor("wqk", [2, 128, 512], BF16, kind="ExternalInput")
    wv_d = nc.dram_tensor("wv", [2, 128, 256], BF16, kind="ExternalInput")
    wp_d = nc.dram_tensor("wp", [2, 128, 256], BF16, kind="ExternalInput")
    wf1_d = nc.dram_tensor("wf1", [2, 128, INNER], BF16, kind="ExternalInput")
    wf2_d = nc.dram_tensor("wf2", [8, 128, 256], BF16, kind="ExternalInput")

    # window-gathered views of x / out:
    # [b, (gh hh), (gw ww), c] -> [b, gh, gw, (hh ww), c]
    x_g = x_d.ap().rearrange("b (gh hh) (gw ww) c -> b gh gw hh ww c",
                             gh=GH, gw=GW)
    out_g = out_d.ap().rearrange("b (gh hh) (gw ww) c -> b gh gw hh ww c",
                                 gh=GH, gw=GW)

    with tile.TileContext(nc) as tc:
        consts = tc.alloc_tile_pool(name="consts", bufs=1)
        pool_x = tc.alloc_tile_pool(name="x", bufs=2)
        pool_ln = tc.alloc_tile_pool(name="ln", bufs=4)
        pool_fm = tc.alloc_tile_pool(name="fm", bufs=6)
        pool_qk = tc.alloc_tile_pool(name="qk", bufs=2)
        pool_v = tc.alloc_tile_pool(name="v", bufs=5)
        pool_e = tc.alloc_tile_pool(name="e", bufs=12)
        pool_ot = tc.alloc_tile_pool(name="ot", bufs=12)
        pool_of = tc.alloc_tile_pool(name="of", bufs=4)
        pool_g = tc.alloc_tile_pool(name="g", bufs=3)
        pool_f2 = tc.alloc_tile_pool(name="f2", bufs=4)
        pool_st = tc.alloc_tile_pool(name="st", bufs=3)
        psum_big = tc.alloc_tile_pool(name="pbig", bufs=2, space="PSUM")
        psum_acc = tc.alloc_tile_pool(name="pacc", bufs=1, space="PSUM")
        psum_sm = tc.alloc_tile_pool(name="psm", bufs=4, space="PSUM")
        psum_tr = tc.alloc_tile_pool(name="ptr", bufs=1, space="PSUM")

        # ---- constants ----
        id128 = consts.tile([128, 128], BF16)
        make_identity(nc, id128)
        eps_sb = consts.tile([128, 1], F32)
        nc.gpsimd.memset(eps_sb, EPS)

        def load_w(dram_ap, n, shape, nm):
            ts = []
            for i in range(n):
                t = consts.tile(shape, BF16, name=f"{nm}{i}")
                nc.sync.dma_start(out=t, in_=dram_ap[i])
                ts.append(t)
            return ts

        wqk_sb = load_w(wqk_d.ap(), 2, [128, 512], "wqk")
        wv_sb = load_w(wv_d.ap(), 2, [128, 256], "wv")
        wp_sb = load_w(wp_d.ap(), 2, [128, 256], "wp")
        wf1_sb = load_w(wf1_d.ap(), 2, [128, INNER], "wf1")
        wf2_sb = load_w(wf2_d.ap(), 8, [128, 256], "wf2")

        NWC = 32            # windows per chunk (half a batch elem)
        NTOKC = NWC * NT    # 2560
        NBLKC = NWC // WBLK  # 8
        GW_W = GH           # hh count per half = NWC // GW_W = 4

        def emit_store(b, hh0, x_wc4):
            hw2 = NWC // GW_W // 2
            for sub in range(2):
                for gh in range(GH):
                    nc.sync.dma_start(
                        out=out_g[b, gh][:, hh0 + sub * hw2:
                                         hh0 + (sub + 1) * hw2],
                        in_=x_wc4[gh * GW:(gh + 1) * GW,
                                  sub * hw2:(sub + 1) * hw2])

        def emit_ln(x_wc, on_act=False):
            """x_wc [80, 64, 256] f32 -> per-token (mean, 1/std as bf16);
            gamma/beta folded into downstream weights. Stats on DVE
            (bn_stats) or ACT (Square/Identity with accum_out)."""
            m = pool_st.tile([80, NWC], F32, tag="m")
            var = pool_st.tile([80, NWC], F32, tag="var")
            t0 = pool_st.tile([80, NWC], F32, tag="t0")
            t1 = pool_st.tile([80, NWC], F32, tag="t1")
            if on_act:
                sums = pool_st.tile([80, NWC], F32, tag="sums")
                sumsq = pool_st.tile([80, NWC], F32, tag="sumsq")
                for w0 in range(NWC):
                    scr = pool_ln.tile([80, C], BF16, tag="scr",
                                       name=f"scr_{w0}")
                    nc.scalar.activation(scr, x_wc[:, w0], AF.Identity,
                                         accum_out=sums[:, w0:w0 + 1])
                    nc.scalar.activation(scr, x_wc[:, w0], AF.Square,
                                         accum_out=sumsq[:, w0:w0 + 1])
                # mean = sum/C ; var = sumsq/C - mean^2
                nc.vector.tensor_scalar(m, sums, 1.0 / C, None, ALU.mult)
                nc.vector.tensor_tensor(t1, m, m, ALU.mult)
                nc.vector.tensor_scalar(t0, sumsq, 1.0 / C, None, ALU.mult)
                nc.vector.tensor_tensor(var, t0, t1, ALU.subtract)
            else:
                st6 = pool_st.tile([80, NWC, 6], F32, tag="st6")
                for w0 in range(NWC):
                    nc.vector.bn_stats(st6[:, w0], x_wc[:, w0])
                # mean = (m_even + m_odd) / 2
                nc.vector.tensor_tensor(t0, st6[:, :, 1], st6[:, :, 4],
                                        ALU.add)
                nc.vector.tensor_scalar(m, t0, 0.5, None, ALU.mult)
                # var = (cv_e + cv_o)/256 + ((m_e - m_o)/2)^2
                nc.vector.tensor_tensor(t0, st6[:, :, 2], st6[:, :, 5],
                                        ALU.add)
                nc.vector.tensor_tensor(t1, st6[:, :, 1], st6[:, :, 4],
                                        ALU.subtract)
                nc.vector.tensor_tensor(t1, t1, t1, ALU.mult)
                nc.vector.tensor_scalar(t0, t0, 1.0 / C, None, ALU.mult)
                nc.vector.tensor_scalar(t1, t1, 0.25, None, ALU.mult)
                nc.vector.tensor_tensor(var, t0, t1, ALU.add)
            # r = rsqrt(var + eps) = exp(-0.5 * ln(var + eps))
            lnv = pool_st.tile([80, NWC], F32, tag="lnv")
            r = pool_st.tile([80, NWC], F32, tag="r")
            rb = pool_st.tile([80, NWC], BF16, tag="rb")
            nc.scalar.activation(lnv, var, AF.Ln, bias=eps_sb[0:80],
                                 scale=1.0)
            nc.scalar.activation(r, lnv, AF.Exp, bias=0.0, scale=-0.5)
            nc.vector.tensor_copy(rb, r)
            return m, rb, r, None

        def emit_apply_transpose(x_wc, lnstats, fm, nm):
            """LN apply (h = (x - m) * r, bf16) in 4-window chunks, then
            per-window PE transposes into fm = [fm0, fm1] [128, 5120].
            Apply rotates across gpsimd/DVE TT pairs and fused per-window
            ACT ops (func(scale*x + bias) with per-partition scale/bias)."""
            m, rb, r, negmr = lnstats
            for g in range(NBLKC):
                h_bf = pool_ln.tile([80, WBLK, C], BF16, tag="h",
                                    name=f"h_{nm}_{g}")
                for wi in range(WBLK):
                    w = g * WBLK + wi
                    # h = (x - m) * r in one fused two-op tensor_scalar
                    eng = nc.vector if w % 3 == 0 else nc.gpsimd
                    eng.tensor_scalar(h_bf[:, wi], x_wc[:, w],
                                      m[:, w:w + 1], r[:, w:w + 1],
                                      ALU.subtract, ALU.mult)
                for ch in range(2):
                    pt = psum_tr.tile([128, BLKTOK], BF16, tag="tr")
                    for wi in range(WBLK):
                        nc.tensor.matmul(
                            pt[:, wi * NT:(wi + 1) * NT],
                            h_bf[:, wi, ch * 128:(ch + 1) * 128],
                            id128[0:80, 0:80],
                            is_transpose=True)
                    dst = fm[ch][:, g * BLKTOK:(g + 1) * BLKTOK]
                    if (g + ch) % 2 == 0:
                        nc.scalar.activation(dst, pt, AF.Copy)
                    else:
                        nc.vector.tensor_copy(dst, pt)

        def emit_chunk(b, half):
            # ---- load x window-gathered (half = 32 windows: hh 4*half..) ----
            hh0 = half * (NWC // GW_W)
            x_wc = pool_x.tile([80, NWC, C], F32, tag="x",
                               name=f"x_{b}_{half}")
            x_wc4 = x_wc.rearrange("p (hh ww) c -> p hh ww c", hh=NWC // GW_W)
            hw2 = NWC // GW_W // 2
            for gh in range(GH):
                for sub in range(2):
                    hs2 = slice(hh0 + sub * hw2, hh0 + (sub + 1) * hw2)
                    nc.gpsimd.dma_start(
                        out=x_wc4[gh * GW:(gh + 1) * GW,
                                  sub * hw2:(sub + 1) * hw2],
                        in_=x_g[b, gh][:, hs2])

            if KSTAGE < 2:
                emit_store(b, hh0, x_wc4)
                return

            # ---- LN1 + transpose to feature-major ----
            ln1 = emit_ln(x_wc)
            hfm = [pool_fm.tile([128, NTOKC], BF16, tag="hfm", name=f"hfm{b}_{half}_{i}")
                   for i in range(2)]
            emit_apply_transpose(x_wc, ln1, hfm, f"b{b}_{half}ln1")

            # ---- QKV: q, k (feature-major) ----
            # qk[0:2] = q tiles (4 heads each), qk[2:4] = k tiles
            qk = [pool_qk.tile([128, NTOKC], BF16, tag=f"qk{i}", name=f"qk{b}_{half}_{i}")
                  for i in range(4)]
            for g in range(NBLKC):
                sl = slice(g * BLKTOK, (g + 1) * BLKTOK)
                for mc in range(4):
                    pq = psum_big.tile([128, BLKTOK], F32, tag="big")
                    for kc in range(2):
                        nc.tensor.matmul(
                            pq, wqk_sb[kc][:, mc * 128:(mc + 1) * 128],
                            hfm[kc][:, sl],
                            start=(kc == 0), stop=(kc == 1))
                    if mc < 2:
                        nc.vector.tensor_copy(qk[mc][:, sl], pq)
                    else:
                        nc.scalar.activation(qk[mc][:, sl], pq, AF.Copy)

            if KSTAGE < 3:
                dummy = pool_ot.tile([80, C], BF16, tag="otm",
                                     name=f"dmy{b}_{half}")
                nc.vector.tensor_copy(dummy[0:64, 0:128],
                                      qk[0][0:64, 0:128])
                nc.vector.tensor_copy(dummy[0:64, 128:256],
                                      hfm[0][0:64, 0:128])
                emit_store(b, hh0, x_wc4)
                return

            # ---- attention + flipped proj + residual1 ----
            # NB: all matmuls writing one PSUM tile must share tile_position,
            # so S' groups by head class c = h % 4 (heads {c, c+4}) across a
            # window triple: 6 window-heads per tile, one position (32c, 0).
            # v (flipped matmuls, + ones column) in window-pairs, emitted
            # on demand just ahead of each attention group (pool-depth bound)
            v33t = {}

            def emit_v_pair(vp):
                wp = vp * 2
                v33 = pool_v.tile([80, 2, HEADS, 33], BF16, tag="v33",
                                  name=f"v33_{b}_{half}_{wp}")
                nc.gpsimd.memset(v33[:, :, :, 32], 1.0)
                pv = psum_sm.tile([80, 2, 256], F32, tag="sm",
                                  name=f"pv_{b}_{half}_{wp}")
                for u in range(2):
                    for kc in range(2):
                        nc.tensor.matmul(
                            pv[:, u],
                            hfm[kc][:, (wp + u) * NT:(wp + u + 1) * NT],
                            wv_sb[kc], start=(kc == 0), stop=(kc == 1))
                dstv = v33[:, :, :, 0:32]
                srcv = pv.rearrange("p u (h d) -> p u h d", h=HEADS)
                if vp % 2 == 0:
                    nc.vector.tensor_copy(dstv, srcv)
                else:
                    nc.scalar.activation(dstv, srcv, AF.Copy)
                v33t[vp] = v33

            ofm = [None, None]
            otp = [None, None]
            NWG = 3  # windows per S' group
            next_vp = 0
            for w0 in range(0, NWC, NWG):
                nw = min(NWG, NWC - w0)
                while next_vp * 2 < w0 + nw:
                    emit_v_pair(next_vp)
                    next_vp += 1
                egs = []
                for c in range(4):
                    ps = psum_sm.tile([80, 160 * NWG], F32, tag="sm",
                                      name=f"ps_{b}_{half}_{w0}_{c}")
                    for j in range(nw):
                        for hh in range(2):
                            h = c + 4 * hh
                            i = 2 * j + hh
                            ts = slice((w0 + j) * NT, (w0 + j + 1) * NT)
                            hs = slice(32 * c, 32 * c + 32)
                            nc.tensor.matmul(
                                ps[:, i * 80:(i + 1) * 80],
                                qk[2 + h // 4][hs, ts], qk[h // 4][hs, ts],
                                tile_position=(32 * c, 0))
                    eg = pool_e.tile([80, 160 * NWG], BF16, tag="e",
                                     name=f"eg_{b}_{half}_{w0}_{c}")
                    nc.scalar.activation(eg[:, :160 * nw], ps[:, :160 * nw],
                                         AF.Exp, bias=0.0, scale=SCALE)
                    egs.append(eg)
                if KATTN < 1:
                    continue

                for w in range(w0, w0 + nw):
                    po = psum_sm.tile([80, HEADS * 33], F32, tag="sm")
                    for h in range(HEADS):
                        c, hh = h % 4, h // 4
                        i = 2 * (w - w0) + hh
                        nc.tensor.matmul(po[:, h * 33:(h + 1) * 33],
                                         egs[c][:, i * 80:(i + 1) * 80],
                                         v33t[w // 2][:, w % 2, h, :])
                    pov = po.rearrange("p (h d) -> p h d", h=HEADS)
                    r8 = pool_st.tile([80, HEADS], F32, tag="r8")
                    nc.vector.reciprocal(r8, pov[:, :, 32])
                    otm = pool_ot.tile([80, C], BF16, tag="otm")
                    nc.vector.tensor_tensor(
                        otm.rearrange("p (h d) -> p h d", h=HEADS),
                        pov[:, :, 0:32],
                        r8[:, :, None].broadcast_to([80, HEADS, 32]),
                        ALU.mult)
                    if KATTN < 2:
                        continue
                    # transpose O into a per-4-window psum group; evict and
                    # run proj + residual once the group is complete
                    wi = w % WBLK
                    if wi == 0:
                        ofm[0] = pool_of.tile([128, BLKTOK], BF16,
                                              tag="of0", name=f"of0_{b}_{half}_{w}")
                        ofm[1] = pool_of.tile([128, BLKTOK], BF16,
                                              tag="of1", name=f"of1_{b}_{half}_{w}")
                        otp[0] = psum_tr.tile([128, 2, BLKTOK], BF16, tag="tr",
                                              name=f"otp_{b}_{half}_{w}")
                    for ch in range(2):
                        nc.tensor.matmul(otp[0][:, ch, wi * NT:(wi + 1) * NT],
                                         otm[:, ch * 128:(ch + 1) * 128],
                                         id128[0:80, 0:80],
                                         is_transpose=True)
                    if wi < WBLK - 1:
                        continue
                    for ch in range(2):
                        nc.scalar.activation(ofm[ch], otp[0][:, ch], AF.Copy)
                    if KATTN < 3:
                        continue
                    for wj in range(0, WBLK, 2):
                        wq = w - (WBLK - 1) + wj
                        pp = psum_sm.tile([80, 2, 256], F32, tag="sm",
                                          name=f"pp_{b}_{half}_{wq}")
                        for u in range(2):
                            for kc in range(2):
                                nc.tensor.matmul(
                                    pp[:, u],
                                    ofm[kc][:, (wj + u) * NT:
                                            (wj + u + 1) * NT],
                                    wp_sb[kc], start=(kc == 0),
                                    stop=(kc == 1))
                        nc.vector.tensor_tensor(x_wc[:, wq:wq + 2],
                                                x_wc[:, wq:wq + 2], pp,
                                                ALU.add)

            if KSTAGE < 4:
                emit_store(b, hh0, x_wc4)
                return

            # ---- LN2 + transpose ----
            ln2 = emit_ln(x_wc)
            h2fm = [pool_fm.tile([128, NTOKC], BF16, tag="hfm",
                                 name=f"h2fm{b}_{half}_{i}") for i in range(2)]
            emit_apply_transpose(x_wc, ln2, h2fm, f"b{b}_{half}ln2")

            # ---- MLP ----
            for g in range(NBLKC):
                sl = slice(g * BLKTOK, (g + 1) * BLKTOK)
                gsb = pool_g.tile([128, 8, BLKTOK], BF16, tag="g")
                for mc in range(8):
                    pf = psum_big.tile([128, BLKTOK], F32, tag="big")
                    for kc in range(2):
                        nc.tensor.matmul(
                            pf, wf1_sb[kc][:, mc * 128:(mc + 1) * 128],
                            h2fm[kc][:, sl],
                            start=(kc == 0), stop=(kc == 1))
                    nc.scalar.activation(gsb[:, mc], pf, AF.Gelu)
                f2 = [pool_f2.tile([128, BLKTOK], BF16, tag=f"f2{mc}",
                                name=f"f2_{b}_{half}_{g}_{mc}") for mc in range(2)]
                for mc in range(2):
                    pa = psum_acc.tile([128, BLKTOK], F32, tag="acc")
                    for kc in range(8):
                        nc.tensor.matmul(
                            pa, wf2_sb[kc][:, mc * 128:(mc + 1) * 128],
                            gsb[:, kc],
                            start=(kc == 0), stop=(kc == 7))
                    nc.vector.tensor_copy(f2[mc], pa)
                # transpose back + residual2 in place
                for mc in range(2):
                    pt = psum_tr.tile([80, WBLK * 128], BF16, tag="tr")
                    for wi in range(WBLK):
                        nc.tensor.matmul(
                            pt[:, wi * 128:(wi + 1) * 128],
                            f2[mc][:, wi * NT:(wi + 1) * NT],
                            id128, is_transpose=True)
                    xsl = x_wc[:, g * WBLK:(g + 1) * WBLK,
                               mc * 128:(mc + 1) * 128]
                    nc.vector.tensor_tensor(
                        xsl, xsl, pt.rearrange("p (w c) -> p w c", w=WBLK),
                        ALU.add)

            # ---- store ----
            emit_store(b, hh0, x_wc4)

        for b in range(B_LOC):
            for half in range(2):
                emit_chunk(b, half)

        for p in reversed((consts, pool_x, pool_ln, pool_fm, pool_qk,
                           pool_v, pool_e, pool_ot, pool_of, pool_g, pool_f2,
                           pool_st, psum_big, psum_acc, psum_sm, psum_tr)):
            p.release()

    nc.compile()
    return nc


_NC_CACHE = None


def _get_nc():
    global _NC_CACHE
    if _NC_CACHE is None:
        _NC_CACHE = build_nc()
    return _NC_CACHE


def _prep_weights(norm1_g, norm1_b, qkv_w, qkv_b, proj_w, proj_b, ls1_g,
                  norm2_g, norm2_b, fc1_w, fc1_b, fc2_w, fc2_b, ls2_g):
    """Host-side weight folding. Returns dict of device weight arrays.

    gamma folds into the following matmul's weights; beta/bias terms must
    be zero (true for this module's init) — asserted here.
    """
    qkv_w = np.asarray(qkv_w, np.float32)
    w_eff = np.asarray(norm1_g, np.float32)[:, None] * qkv_w
    b_eff = np.asarray(norm1_b, np.float32) @ qkv_w + np.asarray(qkv_b)
    f1_eff = np.asarray(norm2_g, np.float32)[:, None] * np.asarray(fc1_w)
    f1b_eff = np.asarray(norm2_b, np.float32) @ np.asarray(fc1_w) + fc1_b
    wp_eff = np.asarray(proj_w, np.float32) * np.asarray(ls1_g)[None, :]
    pb_eff = np.asarray(proj_b) * np.asarray(ls1_g)
    wf2_eff = np.asarray(fc2_w, np.float32) * np.asarray(ls2_g)[None, :]
    f2b_eff = np.asarray(fc2_b) * np.asarray(ls2_g)
    for nm, v in [("qkv_b", b_eff), ("fc1_b", f1b_eff), ("proj_b", pb_eff),
                  ("fc2_b", f2b_eff)]:
        assert np.allclose(np.asarray(v), 0.0, atol=1e-30), \
            f"nonzero {nm} not supported by this kernel build"
    return {
        "wqk": _bf16(w_eff[:, :512]).reshape(2, 128, 512),
        "wv": _bf16(w_eff[:, 512:768]).reshape(2, 128, 256),
        "wp": _bf16(wp_eff).reshape(2, 128, 256),
        "wf1": _bf16(f1_eff).reshape(2, 128, INNER),
        "wf2": _bf16(wf2_eff).reshape(8, 128, 256),
    }


def run_sharded(inputs, trace=False, trace_kwargs=None):
    """inputs: full-problem dict from setup_inputs(). Returns
    (out [B,H,W,C] f32, BassKernelResults)."""
    nc = _get_nc()
    x = np.asarray(inputs["x"], np.float32)
    wmap = _prep_weights(
        inputs["norm1_g"], inputs["norm1_b"], inputs["qkv_w"],
        inputs["qkv_b"], inputs["proj_w"], inputs["proj_b"], inputs["ls1_g"],
        inputs["norm2_g"], inputs["norm2_b"], inputs["fc1_w"],
        inputs["fc1_b"], inputs["fc2_w"], inputs["fc2_b"], inputs["ls2_g"])
    in_maps = []
    for c in range(NCORES):
        m = dict(wmap)
        m["x"] = np.ascontiguousarray(x[c * B_LOC:(c + 1) * B_LOC])
        in_maps.append(m)
    kw = {}
    if trace:
        kw["trace"] = True
        kw["trace_kwargs"] = trace_kwargs or {}
    res = bass_utils.run_bass_kernel_spmd(nc, in_maps,
                                          core_ids=list(range(NCORES)), **kw)
    out = np.concatenate([res.results[c]["out"] for c in range(NCORES)],
                         axis=0)
    return out, res


def kernel(**inputs) -> np.ndarray:
    out, _ = run_sharded(inputs)
    return out.astype(np.float32)


if __name__ == "__main__":
    nc = build_nc()
    print("built + compiled ok")
